# revision 1
# baseline (speedup 1.0000x reference)
"""Sparse (sliding-window) attention head on 8 TRN2 NeuronCores.

Reference computation (B=2, S=4096, D=512, HD=64, SCALE=128):
    q = x @ wq ; k = x @ wk ; v = x @ wv          [B,S,64]
    scores[b,s,w] = q[b,s] . k[b,s-128+w] / 8     w in [0,256), zero-padded OOB
    out = softmax_w(scores) @ v_window            [B,S,64]

Sharding: 8 shards = (batch b, 1024-seq chunk c). Each shard gets a
zero-padded 128-halo of x on both sides, which reproduces the reference's
zero-padded (not masked) window semantics exactly. All compute is local,
no collectives.

Device layout (per core):
    xT   [512,1280] bf16  host-pre-transposed padded input shard
    w3   [128,3,4,64] bf16  packed wq|wk|wv (d-chunk on partitions)
    mask [128,384] bf16  band-validity mask for one 128-query block
    out  [1024,64] f32

    qT,kT = w.T @ xT   (head dim on partitions)
    v     = xT.T @ wv  (natural layout, keys on partitions) + ones column
    per 128-query block qb:
        scT[key,que] = kT_chunk.T @ qT_block      3 chunks of [128,128]
        e = exp(scT/8) * mask                     bf16
        av[que,0:65] = sum_c e_c.T @ vaug_c       ones col -> softmax denom
        out_block = av[:, :64] * (1/av[:, 64])
"""

import sys
import types

import numpy as np
import ml_dtypes

B, S, D = 2, 4096, 512
HD = 64
SCALE = 128
SS = S // 4          # 1024 positions per shard
HP = SCALE           # halo padding each side
NP = SS + 2 * HP     # 1280 padded positions
NKC = NP // 128      # 10 key chunks
NQB = SS // 128      # 8 query blocks
NDC = D // 128       # 4 d-chunks

_CACHE = {}


def _ensure_hooks():
    """Register the axon NTFF profile hook; keep artifacts local."""
    if "antenv.axon_hooks" not in sys.modules:
        try:
            from trn_agent_boot.trn_boot import _ntff_profile_via_ctypes

            m = types.ModuleType("antenv.axon_hooks")
            m.get_axon_ntff_profile_hook = lambda: _ntff_profile_via_ctypes(
                "/opt/axon/libaxon_pjrt.so"
            )
            sys.modules["antenv.axon_hooks"] = m
        except Exception:
            pass
    import concourse.bass_utils as bass_utils

    bass_utils.upload_artifacts = lambda tmpdir: tmpdir


def _build_nc():
    import concourse.mybir as mybir
    import concourse.tile as tile
    from concourse import bacc

    bf = mybir.dt.bfloat16
    f32 = mybir.dt.float32
    AF = mybir.ActivationFunctionType

    nc = bacc.Bacc("TRN2", target_bir_lowering=False, debug=False, num_devices=8)

    xT_d = nc.dram_tensor("xT", [D, NP], bf, kind="ExternalInput")
    w3_d = nc.dram_tensor("w3", [128, 3, NDC, HD], bf, kind="ExternalInput")
    mask_d = nc.dram_tensor("mask", [128, 384], bf, kind="ExternalInput")
    out_d = nc.dram_tensor("out", [SS, HD], f32, kind="ExternalOutput")

    with tile.TileContext(nc) as tc:
        with (
            tc.tile_pool(name="consts", bufs=1) as consts,
            tc.tile_pool(name="xtp", bufs=1) as xtp,
            tc.tile_pool(name="qkp", bufs=1) as qkp,
            tc.tile_pool(name="vgp", bufs=1) as vgp,
            tc.tile_pool(name="work", bufs=3) as work,
            tc.tile_pool(name="fin", bufs=3) as fin,
        ):
            w_s = consts.tile([128, 3, NDC, HD], bf)
            nc.sync.dma_start(out=w_s, in_=w3_d[:, :, :, :])
            mask_s = consts.tile([128, 384], bf)
            nc.sync.dma_start(out=mask_s, in_=mask_d[:, :])

            # Trigger the ACT exp table load early so it hides under DMA/proj.
            zz = consts.tile([128, 1], f32)
            nc.vector.memset(zz, 0.0)
            ez = consts.tile([128, 1], f32)
            nc.scalar.activation(ez, zz, AF.Exp)

            xt = []
            for dc in range(NDC):
                t = xtp.tile([128, NP], bf, tag=f"xt{dc}")
                nc.sync.dma_start(out=t, in_=xT_d[dc * 128 : (dc + 1) * 128, :])
                xt.append(t)

            qT_s = qkp.tile([64, SS], bf, tag="qT")
            kT_s = qkp.tile([64, NP], bf, tag="kT")
            vaug = vgp.tile([128, NKC, 66], bf)
            nc.vector.memset(vaug[:, :, 64:66], 1.0)

            with (
                tc.tile_pool(name="pps", bufs=2, space="PSUM") as pps,
                tc.tile_pool(name="vps", bufs=2, space="PSUM") as vps,
            ):
                # qT / kT projections: head dim on partitions.
                segs_q = [(0, 512), (512, 512)]
                segs_k = [(0, 512), (512, 512), (1024, 256)]
                for j, dst, off, segs in ((0, qT_s, HP, segs_q), (1, kT_s, 0, segs_k)):
                    for s0, w in segs:
                        ps = pps.tile([64, 512], f32, tag="pp")
                        for dc in range(NDC):
                            nc.tensor.matmul(
                                ps[:, :w],
                                lhsT=w_s[:, j, dc, :],
                                rhs=xt[dc][:, off + s0 : off + s0 + w],
                                start=(dc == 0),
                                stop=(dc == NDC - 1),
                            )
                        nc.scalar.copy(dst[:, s0 : s0 + w], ps[:, :w])

                # v in natural layout (keys on partitions) + ones column.
                for kc in range(NKC):
                    vp = vps.tile([128, HD], f32, tag="vp")
                    for dc in range(NDC):
                        nc.tensor.matmul(
                            vp,
                            lhsT=xt[dc][:, kc * 128 : (kc + 1) * 128],
                            rhs=w_s[:, 2, dc, :],
                            start=(dc == 0),
                            stop=(dc == NDC - 1),
                        )
                    nc.vector.tensor_copy(vaug[:, kc, 0:64], vp)

            with (
                tc.tile_pool(name="sps", bufs=2, space="PSUM") as sps,
                tc.tile_pool(name="aps", bufs=2, space="PSUM") as aps,
            ):
                for qb in range(NQB):
                    sc = sps.tile([128, 384], f32, tag="sc")
                    for c in range(3):
                        nc.tensor.matmul(
                            sc[:, c * 128 : (c + 1) * 128],
                            lhsT=kT_s[:, (qb + c) * 128 : (qb + c + 1) * 128],
                            rhs=qT_s[:, qb * 128 : (qb + 1) * 128],
                            start=True,
                            stop=True,
                        )
                    ex = work.tile([128, 384], bf, tag="ex")
                    nc.scalar.activation(ex, sc, AF.Exp, scale=0.125)
                    em = work.tile([128, 384], bf, tag="em")
                    nc.vector.tensor_mul(em, ex, mask_s)
                    av = aps.tile([128, 65], f32, tag="av")
                    for c in range(3):
                        nc.tensor.matmul(
                            av,
                            lhsT=em[:, c * 128 : (c + 1) * 128],
                            rhs=vaug[:, qb + c, 0:65],
                            start=(c == 0),
                            stop=(c == 2),
                        )
                    rc = fin.tile([128, 1], f32, tag="rc")
                    nc.vector.reciprocal(rc, av[:, 64:65])
                    ot = fin.tile([128, HD], f32, tag="ot")
                    nc.scalar.activation(ot, av[:, 0:HD], AF.Copy, scale=rc)
                    nc.sync.dma_start(
                        out=out_d[qb * 128 : (qb + 1) * 128, :], in_=ot
                    )

    nc.compile()
    return nc


def _get_nc():
    if "nc" not in _CACHE:
        _ensure_hooks()
        _CACHE["nc"] = _build_nc()
    return _CACHE["nc"]


def _host_inputs(inputs, wq, wk, wv):
    bf16 = ml_dtypes.bfloat16
    x = np.asarray(inputs, dtype=np.float32)

    # w3[p, j, c, m] = w_j[c*128 + p, m]
    w3 = np.stack([wq, wk, wv]).astype(np.float32)          # [3, 512, 64]
    w3 = w3.reshape(3, NDC, 128, HD).transpose(2, 0, 1, 3)   # [128, 3, 4, 64]
    w3 = np.ascontiguousarray(w3).astype(bf16)

    p = np.arange(128)[:, None]
    q = np.arange(128)[None, :]
    mask = np.concatenate(
        [(p >= q), np.ones((128, 128), bool), (p < q)], axis=1
    ).astype(bf16)                                           # [128, 384]

    in_maps = []
    for i in range(8):
        b, c = divmod(i, 4)
        s0 = c * SS
        xp = np.zeros((NP, D), np.float32)
        lo = max(0, s0 - HP)
        hi = min(S, s0 + SS + HP)
        xp[lo - (s0 - HP) : hi - (s0 - HP)] = x[b, lo:hi]
        xT = np.ascontiguousarray(xp.T).astype(bf16)         # [512, 1280]
        in_maps.append({"xT": xT, "w3": w3, "mask": mask})
    return in_maps


def run_sharded(inputs, wq, wk, wv, trace=False, trace_cores=None):
    """Run the SPMD kernel; returns (out [B,S,HD] f32, BassKernelResults)."""
    _ensure_hooks()
    import concourse.bass_utils as bass_utils

    nc = _get_nc()
    in_maps = _host_inputs(inputs, wq, wk, wv)
    res = bass_utils.run_bass_kernel_spmd(
        nc,
        in_maps,
        core_ids=list(range(8)),
        trace=trace,
        trace_cores=trace_cores,
    )
    out = np.empty((B, S, HD), np.float32)
    for i in range(8):
        b, c = divmod(i, 4)
        out[b, c * SS : (c + 1) * SS] = res.results[i]["out"]
    return out, res


def kernel(inputs, wq, wk, wv):
    out, _ = run_sharded(inputs, wq, wk, wv, trace=False)
    return out


# revision 5
# speedup vs baseline: 1.1085x; 1.1085x over previous
"""Sparse (sliding-window) attention head on 8 TRN2 NeuronCores.

Reference computation (B=2, S=4096, D=512, HD=64, SCALE=128):
    q = x @ wq ; k = x @ wk ; v = x @ wv          [B,S,64]
    scores[b,s,w] = q[b,s] . k[b,s-128+w] / 8     w in [0,256), zero-padded OOB
    out = softmax_w(scores) @ v_window            [B,S,64]

Sharding: 8 shards = (batch b, 1024-seq chunk c). Each shard gets a
zero-padded 128-halo of x on both sides, which reproduces the reference's
zero-padded (not masked) window semantics exactly. All compute is local,
no collectives.

Device layout (per core):
    xT   [512,1280] bf16  host-pre-transposed padded input shard
    w3   [128,3,4,64] bf16  packed wq|wk|wv (d-chunk on partitions)
    mask [128,384] bf16  band-validity mask for one 128-query block
    out  [1024,64] f32

    qT,kT = w.T @ xT   (head dim on partitions)
    v     = xT.T @ wv  (natural layout, keys on partitions) + ones column
    per 128-query block qb:
        scT[key,que] = kT_chunk.T @ qT_block      3 chunks of [128,128]
        e = exp(scT/8) * mask                     bf16
        av[que,0:65] = sum_c e_c.T @ vaug_c       ones col -> softmax denom
        out_block = av[:, :64] * (1/av[:, 64])
"""

import sys
import types

import numpy as np
import ml_dtypes

B, S, D = 2, 4096, 512
HD = 64
SCALE = 128
SS = S // 4          # 1024 positions per shard
HP = SCALE           # halo padding each side
NP = SS + 2 * HP     # 1280 padded positions
NKC = NP // 128      # 10 key chunks
NQB = SS // 128      # 8 query blocks
NDC = D // 128       # 4 d-chunks

_CACHE = {}


def _ensure_hooks():
    """Register the axon NTFF profile hook; keep artifacts local."""
    if "antenv.axon_hooks" not in sys.modules:
        try:
            from trn_agent_boot.trn_boot import _ntff_profile_via_ctypes

            m = types.ModuleType("antenv.axon_hooks")
            m.get_axon_ntff_profile_hook = lambda: _ntff_profile_via_ctypes(
                "/opt/axon/libaxon_pjrt.so"
            )
            sys.modules["antenv.axon_hooks"] = m
        except Exception:
            pass
    import concourse.bass_utils as bass_utils

    bass_utils.upload_artifacts = lambda tmpdir: tmpdir


def _build_nc():
    import concourse.mybir as mybir
    import concourse.tile as tile
    from concourse import bacc

    bf = mybir.dt.bfloat16
    f32 = mybir.dt.float32
    AF = mybir.ActivationFunctionType

    nc = bacc.Bacc("TRN2", target_bir_lowering=False, debug=False, num_devices=8)

    xT_d = nc.dram_tensor("xT", [D, NP], bf, kind="ExternalInput")
    w3_d = nc.dram_tensor("w3", [128, 3, NDC, HD], bf, kind="ExternalInput")
    mask_d = nc.dram_tensor("mask", [128, 384], bf, kind="ExternalInput")
    out_d = nc.dram_tensor("out", [SS, HD], f32, kind="ExternalOutput")

    # xT columns split into chunks so compute can start after the first
    # chunk's DMA instead of the whole 1.3MB load.
    CW = [512, 512, 256]
    COFF = [0, 512, 1024]
    # per chunk: (xT col range) for kT and qT segments, chunk-aligned
    QSEG = [(128, 384), (512, 512), (1024, 128)]  # (xT col start, width)

    with tile.TileContext(nc) as tc:
        with (
            tc.tile_pool(name="consts", bufs=1) as consts,
            tc.tile_pool(name="xtp", bufs=1) as xtp,
            tc.tile_pool(name="qkp", bufs=1) as qkp,
            tc.tile_pool(name="vgp", bufs=1) as vgp,
            tc.tile_pool(name="work", bufs=3) as work,
            tc.tile_pool(name="fin", bufs=3) as fin,
        ):
            w_s = consts.tile([128, 3, NDC, HD], bf)
            nc.sync.dma_start(out=w_s, in_=w3_d[:, :, :, :])
            mask_s = consts.tile([128, 384], bf)
            nc.sync.dma_start(out=mask_s, in_=mask_d[:, :])

            # Trigger the ACT exp table load early so it hides under DMA/proj.
            zz = consts.tile([128, 1], f32)
            nc.vector.memset(zz, 0.0)
            ez = consts.tile([128, 1], f32)
            nc.scalar.activation(ez, zz, AF.Exp)

            xt = {}
            for ch in range(3):
                for dc in range(NDC):
                    t = xtp.tile([128, CW[ch]], bf, tag=f"xt{dc}_{ch}")
                    nc.sync.dma_start(
                        out=t,
                        in_=xT_d[
                            dc * 128 : (dc + 1) * 128,
                            COFF[ch] : COFF[ch] + CW[ch],
                        ],
                    )
                    xt[(dc, ch)] = t

            qT_s = qkp.tile([64, SS], bf, tag="qT")
            kT_s = qkp.tile([64, NP], bf, tag="kT")
            vaug = vgp.tile([128, NKC, 66], bf)
            nc.vector.memset(vaug[:, :, 64:66], 1.0)

            with (
                tc.tile_pool(name="wrm", bufs=1, space="PSUM") as wrm,
                tc.tile_pool(name="pps", bufs=3, space="PSUM") as pps,
                tc.tile_pool(name="vps", bufs=3, space="PSUM") as vps,
            ):
                # PE warmup: ~4.5us of dummy matmuls on w_s while xT DMAs,
                # so HAM un-throttles (1.2 -> 2.4 GHz) before the real work.
                wps = wrm.tile([64, 512], f32, tag="warm")
                for _ in range(9):
                    nc.tensor.matmul(
                        wps,
                        lhsT=w_s[:, 0, 0, :],
                        rhs=w_s[:, 0:2, :, :],
                        start=True,
                        stop=True,
                    )

                # qT / kT projections: head dim on partitions. kT evacs on
                # ACT, qT on DVE to split the PSUM-drain load.
                for ch in range(3):
                    ps = pps.tile([64, 512], f32, tag="pp")
                    for dc in range(NDC):
                        nc.tensor.matmul(
                            ps[:, : CW[ch]],
                            lhsT=w_s[:, 1, dc, :],
                            rhs=xt[(dc, ch)][:, :],
                            start=(dc == 0),
                            stop=(dc == NDC - 1),
                        )
                    nc.scalar.copy(
                        kT_s[:, COFF[ch] : COFF[ch] + CW[ch]], ps[:, : CW[ch]]
                    )

                    qs0, qw = QSEG[ch]
                    lo = qs0 - COFF[ch]
                    ps2 = pps.tile([64, 512], f32, tag="pp")
                    for dc in range(NDC):
                        nc.tensor.matmul(
                            ps2[:, :qw],
                            lhsT=w_s[:, 0, dc, :],
                            rhs=xt[(dc, ch)][:, lo : lo + qw],
                            start=(dc == 0),
                            stop=(dc == NDC - 1),
                        )
                    nc.vector.tensor_copy(
                        qT_s[:, qs0 - HP : qs0 - HP + qw], ps2[:, :qw]
                    )

                # v in natural layout (keys on partitions) + ones column.
                for kc in range(NKC):
                    ch = kc // 4
                    lc = kc * 128 - COFF[ch]
                    vp = vps.tile([128, HD], f32, tag="vp")
                    for dc in range(NDC):
                        nc.tensor.matmul(
                            vp,
                            lhsT=xt[(dc, ch)][:, lc : lc + 128],
                            rhs=w_s[:, 2, dc, :],
                            start=(dc == 0),
                            stop=(dc == NDC - 1),
                        )
                    nc.vector.tensor_copy(vaug[:, kc, 0:64], vp)

            with (
                tc.tile_pool(name="sps", bufs=2, space="PSUM") as sps,
                tc.tile_pool(name="aps", bufs=2, space="PSUM") as aps,
            ):
                for qb in range(NQB):
                    sc = sps.tile([128, 384], f32, tag="sc")
                    for c in range(3):
                        nc.tensor.matmul(
                            sc[:, c * 128 : (c + 1) * 128],
                            lhsT=kT_s[:, (qb + c) * 128 : (qb + c + 1) * 128],
                            rhs=qT_s[:, qb * 128 : (qb + 1) * 128],
                            start=True,
                            stop=True,
                        )
                    ex = work.tile([128, 384], bf, tag="ex")
                    nc.scalar.activation(ex, sc, AF.Exp, scale=0.125)
                    em = work.tile([128, 384], bf, tag="em")
                    nc.vector.tensor_mul(em, ex, mask_s)
                    av = aps.tile([128, 65], f32, tag="av")
                    for c in range(3):
                        nc.tensor.matmul(
                            av,
                            lhsT=em[:, c * 128 : (c + 1) * 128],
                            rhs=vaug[:, qb + c, 0:65],
                            start=(c == 0),
                            stop=(c == 2),
                        )
                    rc = fin.tile([128, 1], f32, tag="rc")
                    nc.vector.reciprocal(rc, av[:, 64:65])
                    ot = fin.tile([128, HD], f32, tag="ot")
                    nc.vector.tensor_scalar_mul(ot, av[:, 0:HD], rc)
                    nc.sync.dma_start(
                        out=out_d[qb * 128 : (qb + 1) * 128, :], in_=ot
                    )

    nc.compile()
    return nc


def _get_nc():
    if "nc" not in _CACHE:
        _ensure_hooks()
        _CACHE["nc"] = _build_nc()
    return _CACHE["nc"]


def _host_inputs(inputs, wq, wk, wv):
    bf16 = ml_dtypes.bfloat16
    x = np.asarray(inputs, dtype=np.float32)

    # w3[p, j, c, m] = w_j[c*128 + p, m]
    w3 = np.stack([wq, wk, wv]).astype(np.float32)          # [3, 512, 64]
    w3 = w3.reshape(3, NDC, 128, HD).transpose(2, 0, 1, 3)   # [128, 3, 4, 64]
    w3 = np.ascontiguousarray(w3).astype(bf16)

    p = np.arange(128)[:, None]
    q = np.arange(128)[None, :]
    mask = np.concatenate(
        [(p >= q), np.ones((128, 128), bool), (p < q)], axis=1
    ).astype(bf16)                                           # [128, 384]

    in_maps = []
    for i in range(8):
        b, c = divmod(i, 4)
        s0 = c * SS
        xp = np.zeros((NP, D), np.float32)
        lo = max(0, s0 - HP)
        hi = min(S, s0 + SS + HP)
        xp[lo - (s0 - HP) : hi - (s0 - HP)] = x[b, lo:hi]
        xT = np.ascontiguousarray(xp.T).astype(bf16)         # [512, 1280]
        in_maps.append({"xT": xT, "w3": w3, "mask": mask})
    return in_maps


def run_sharded(inputs, wq, wk, wv, trace=False, trace_cores=None):
    """Run the SPMD kernel; returns (out [B,S,HD] f32, BassKernelResults)."""
    _ensure_hooks()
    import concourse.bass_utils as bass_utils

    nc = _get_nc()
    in_maps = _host_inputs(inputs, wq, wk, wv)
    res = bass_utils.run_bass_kernel_spmd(
        nc,
        in_maps,
        core_ids=list(range(8)),
        trace=trace,
        trace_cores=trace_cores,
    )
    out = np.empty((B, S, HD), np.float32)
    for i in range(8):
        b, c = divmod(i, 4)
        out[b, c * SS : (c + 1) * SS] = res.results[i]["out"]
    return out, res


def kernel(inputs, wq, wk, wv):
    out, _ = run_sharded(inputs, wq, wk, wv, trace=False)
    return out
